# revision 1
# baseline (speedup 1.0000x reference)
"""GCN (2-layer graph convolution) on 8 TRN2 NeuronCores.

Strategy (1D graph partition):
  - Nodes sharded across 8 cores (12500 rows each); edges partitioned by
    destination row so segment_sum is core-local.
  - Layer 1: each core computes support1 = x_k @ W1 (bf16), AllGather ->
    full table T1 [100000, 128] bf16.
  - SpMM via dma_gather (4 SWDGE queues) of 256B rows + selection-matrix
    matmul segment-sum: per 128-edge chunk, S[e,d] = val[e]*(row[e]==d).
    S is precomputed on the host (it depends only on edge structure) and
    streamed in as sequential DMA, so no per-chunk vector-engine work.
  - h^T = Relu(psum + b1) on ACT (bias along partitions), support2 = h@W2
    one matmul per tile, AllGather -> T2 [100000, 128] bf16 (cols 0:32 used).
  - SpMM2 same way (rhs width 32), then +b2 and log_softmax epilogue.
  - Edges bucketed by col into 4 segments of 25000 so indices fit int16.
"""

import sys

sys.path.insert(0, "/opt/trn_rl_repo")

import numpy as np
import ml_dtypes

import concourse.bass as bass
import concourse.tile as tile
from concourse import bacc, mybir
from concourse.bass_utils import run_bass_kernel_spmd
from concourse.library_config import mlp

N = 100000
E = 3200000
F_IN, F_HID, F_OUT = 512, 128, 32
NC = 8
SHARD = N // NC          # 12500
P = 128
NT = (SHARD + P - 1) // P   # 98 tiles; last has 84 rows
NSEG = 4
SEG = N // NSEG          # 25000 rows per gather segment (fits int16 indices)
QSEG = SHARD // NSEG     # 3125: per-core sub-shard contributed to segment s
BF16 = ml_dtypes.bfloat16


def _seg_of(col):
    """Segment s holds rows {k*SHARD + s*QSEG .. +QSEG} of every core k,
    re-indexed as k*QSEG + (r % QSEG). This lets AllGather #s complete as
    soon as every core finished quarter s of its shard."""
    r = col % SHARD
    return r // QSEG


def _idx_of(col):
    return (col // SHARD) * QSEG + (col % QSEG)


def _preprocess(edge_row, edge_col, edge_val):
    """Sort/pad edges into per-(core, dst-tile, col-segment) buckets of
    whole 128-edge chunks (chunk counts identical across cores), and build
    the per-chunk selection matrices S[e, d] = val[e] * (row_local[e] == d).
    """
    er = edge_row.astype(np.int64)
    ec = edge_col.astype(np.int64)
    k = er // SHARD
    t = (er % SHARD) // P
    s = _seg_of(ec)
    key = (k * NT + t) * NSEG + s
    order = np.argsort(key, kind="stable")
    counts = np.bincount(key, minlength=NC * NT * NSEG).reshape(NC, NT, NSEG)
    C_ts = -(-counts.max(axis=0) // P)          # [NT, NSEG] chunks per bucket
    TC = int(C_ts.sum())
    off_flat = np.concatenate([[0], np.cumsum(C_ts.flatten())])[:-1]
    off_ts = off_flat.reshape(NT, NSEG)

    starts = np.zeros(NC * NT * NSEG + 1, np.int64)
    starts[1:] = np.cumsum(counts.flatten())
    key_s = key[order]
    rank = np.arange(E, dtype=np.int64) - starts[key_s]
    base_kts = (np.arange(NC)[:, None, None] * TC + off_ts[None]) * P
    slot = base_kts.reshape(-1)[key_s] + rank

    idx_slots = np.zeros(NC * TC * P, np.int16)
    rows_slots = np.zeros(NC * TC * P, np.int64)
    vals_slots = np.zeros(NC * TC * P, np.float32)
    ero, eco, evo = edge_row[order], edge_col[order], edge_val[order]
    idx_slots[slot] = _idx_of(eco).astype(np.int16)
    rows_slots[slot] = (ero % SHARD) % P
    vals_slots[slot] = evo.astype(np.float32)

    idx_slots = idx_slots.reshape(NC, TC * P)
    idx16 = np.stack(
        [np.tile(idx_slots[c].reshape(-1, 16).T, (8, 1)) for c in range(NC)]
    )                                                     # [NC, 128, 8*TC]

    # S matrices: smat[core][p, ci*128 + d] = S_chunk_ci[p, d]
    smats = []
    rows_k = rows_slots.reshape(NC, TC * P)
    vals_k = vals_slots.reshape(NC, TC * P)
    for c in range(NC):
        S = np.zeros((TC * P, P), dtype=BF16)
        S[np.arange(TC * P), rows_k[c]] = vals_k[c].astype(BF16)
        smats.append(np.ascontiguousarray(
            S.reshape(TC, P, P).transpose(1, 0, 2).reshape(P, TC * P)))
    return C_ts, off_ts, TC, idx16, smats


def _build_program(C_ts, off_ts, TC):
    f32, bf16, i16 = mybir.dt.float32, mybir.dt.bfloat16, mybir.dt.int16
    nc = bacc.Bacc("TRN2", target_bir_lowering=False, debug=False,
                   num_devices=NC, num_swdge_queues=4)

    xT = nc.dram_tensor("xT", [F_IN, SHARD], bf16, kind="ExternalInput")
    W1b = nc.dram_tensor("W1b", [F_IN, F_HID], bf16, kind="ExternalInput")
    W2b = nc.dram_tensor("W2b", [F_HID, F_OUT], bf16, kind="ExternalInput")
    b1c = nc.dram_tensor("b1c", [P, 1], f32, kind="ExternalInput")
    b2bc = nc.dram_tensor("b2bc", [P, F_OUT], f32, kind="ExternalInput")
    idx16 = nc.dram_tensor("idx16", [P, 8 * TC], i16, kind="ExternalInput")
    smat = nc.dram_tensor("smat", [P, TC * P], bf16, kind="ExternalInput")
    out = nc.dram_tensor("out", [SHARD, F_OUT], f32, kind="ExternalOutput")

    T1_local = nc.dram_tensor("T1_local", [SHARD, F_HID], bf16)
    T1_seg = [nc.dram_tensor(f"T1_seg{s}", [SEG, F_HID], bf16,
                             addr_space="Shared") for s in range(NSEG)]
    T2_local = nc.dram_tensor("T2_local", [SHARD, P], bf16)
    T2_seg = [nc.dram_tensor(f"T2_seg{s}", [SEG, P], bf16,
                             addr_space="Shared") for s in range(NSEG)]

    C_t = C_ts.sum(axis=1)          # chunks per tile
    CMAX = int(C_ts.max())
    CTM = int(C_t.max())

    with tile.TileContext(nc) as tc:
        with (
            tc.tile_pool(name="const", bufs=1) as cpool,
            tc.tile_pool(name="xa", bufs=3) as xapool,
            tc.tile_pool(name="s1o", bufs=3) as s1pool,
            tc.tile_pool(name="g", bufs=16) as gpool,
            tc.tile_pool(name="sm", bufs=16) as smpool,
            tc.tile_pool(name="meta", bufs=6) as mpool,
            tc.tile_pool(name="ep", bufs=8) as eppool,
            tc.tile_pool(name="pa", bufs=1, space="PSUM") as papool,
            tc.tile_pool(name="ph", bufs=2, space="PSUM") as phpool,
            tc.tile_pool(name="ps", bufs=2, space="PSUM") as pspool,
        ):
            nc.gpsimd.load_library(mlp)

            # ---- resident constants ----
            W1_sb = cpool.tile([P, 4, F_HID], bf16)
            nc.sync.dma_start(
                W1_sb[:], W1b.ap().rearrange("(kk p) f -> p kk f", p=P))
            W2_sb = cpool.tile([P, F_OUT], bf16)
            nc.sync.dma_start(W2_sb[:], W2b.ap())
            b1_sb = cpool.tile([P, 1], f32)
            nc.sync.dma_start(b1_sb[:], b1c.ap())
            b2_sb = cpool.tile([P, F_OUT], f32)
            nc.sync.dma_start(b2_sb[:], b2bc.ap())

            # ---- phase A: support1 = x_k @ W1 -> T1_local (bf16) ----
            for m in range(NT):
                m0 = m * P
                mw = min(P, SHARD - m0)
                xa = xapool.tile([P, 4, P], bf16)
                nc.sync.dma_start(
                    xa[:, :, :mw],
                    xT.ap()[:, m0:m0 + mw].rearrange("(kk p) m -> p kk m", p=P))
                ps = papool.tile([P, F_HID], f32, space="PSUM")
                for kk in range(4):
                    nc.tensor.matmul(ps[:mw, :], xa[:, kk, :mw], W1_sb[:, kk, :],
                                     start=(kk == 0), stop=(kk == 3))
                s1 = s1pool.tile([P, F_HID], bf16)
                nc.scalar.activation(s1[:mw, :], ps[:mw, :],
                                     mybir.ActivationFunctionType.Copy)
                nc.sync.dma_start(T1_local.ap()[m0:m0 + mw, :], s1[:mw, :])

            # ---- AllGather T1 (segment-wise, pipelined) ----
            for s in range(NSEG):
                nc.gpsimd.collective_compute(
                    "AllGather", mybir.AluOpType.bypass,
                    replica_groups=[list(range(NC))],
                    ins=[T1_local.ap()[s * QSEG:(s + 1) * QSEG, :].opt()],
                    outs=[T1_seg[s].ap().opt()],
                )

            # ---- phase B: SpMM1 + Relu + @W2 -> T2_local ----
            for t in range(NT):
                t0 = t * P
                tw = min(P, SHARD - t0)
                ct = int(C_t[t])
                coff = int(off_ts[t, 0])
                ix = mpool.tile([P, 8 * CMAX * NSEG], i16, tag="ix")
                nc.sync.dma_start(ix[:, :8 * ct],
                                  idx16.ap()[:, 8 * coff:8 * (coff + ct)])

                ph = phpool.tile([P, P], f32, space="PSUM")
                ci = 0
                for s in range(NSEG):
                    cs = int(C_ts[t, s])
                    if cs == 0:
                        continue
                    boff = int(off_ts[t, s])
                    local_off = boff - coff
                    g = gpool.tile([P, CMAX, P], bf16, tag="g")
                    nc.gpsimd.dma_gather(
                        g[:, :cs, :],
                        T1_seg[s].ap(),
                        ix[:, 8 * local_off:8 * (local_off + cs)],
                        cs * P, cs * P, F_HID,
                        single_packet=False, queue_num=s,
                    )
                    sm = smpool.tile([P, CMAX, P], bf16, tag="sm")
                    nc.sync.dma_start(
                        sm[:, :cs, :], smat.ap()[:, boff * P:(boff + cs) * P])
                    for c in range(cs):
                        nc.tensor.matmul(ph[:], g[:, c, :], sm[:, c, :],
                                         start=(ci == 0), stop=(ci == ct - 1))
                        ci += 1
                # h^T = relu(ph + b1) ; support2 = h @ W2
                hT = eppool.tile([P, P], bf16, tag="hT")
                nc.scalar.activation(hT[:], ph[:],
                                     mybir.ActivationFunctionType.Relu,
                                     bias=b1_sb[:])
                ps2 = pspool.tile([P, F_OUT], f32, space="PSUM")
                nc.tensor.matmul(ps2[:], hT[:], W2_sb[:], start=True, stop=True)
                s2 = eppool.tile([P, F_OUT], bf16, tag="s2")
                nc.vector.tensor_copy(s2[:], ps2[:])
                nc.sync.dma_start(T2_local.ap()[t0:t0 + tw, :F_OUT], s2[:tw, :])

            # ---- AllGather T2 (segment-wise, pipelined) ----
            for s in range(NSEG):
                nc.gpsimd.collective_compute(
                    "AllGather", mybir.AluOpType.bypass,
                    replica_groups=[list(range(NC))],
                    ins=[T2_local.ap()[s * QSEG:(s + 1) * QSEG, :].opt()],
                    outs=[T2_seg[s].ap().opt()],
                )

            # ---- phase D: SpMM2 + bias + log_softmax -> out ----
            for t in range(NT):
                t0 = t * P
                tw = min(P, SHARD - t0)
                ct = int(C_t[t])
                coff = int(off_ts[t, 0])
                ix = mpool.tile([P, 8 * CMAX * NSEG], i16, tag="ix")
                nc.sync.dma_start(ix[:, :8 * ct],
                                  idx16.ap()[:, 8 * coff:8 * (coff + ct)])

                pl = pspool.tile([P, F_OUT], f32, space="PSUM", tag="pl")
                ci = 0
                for s in range(NSEG):
                    cs = int(C_ts[t, s])
                    if cs == 0:
                        continue
                    boff = int(off_ts[t, s])
                    local_off = boff - coff
                    g = gpool.tile([P, CMAX, P], bf16, tag="g")
                    nc.gpsimd.dma_gather(
                        g[:, :cs, :],
                        T2_seg[s].ap(),
                        ix[:, 8 * local_off:8 * (local_off + cs)],
                        cs * P, cs * P, P,
                        single_packet=False, queue_num=s,
                    )
                    sm = smpool.tile([P, CMAX, P], bf16, tag="sm")
                    nc.sync.dma_start(
                        sm[:, :cs, :], smat.ap()[:, boff * P:(boff + cs) * P])
                    for c in range(cs):
                        nc.tensor.matmul(pl[:], sm[:, c, :],
                                         g[:, c, :F_OUT],
                                         start=(ci == 0), stop=(ci == ct - 1))
                        ci += 1
                # logits = pl + b2 ; out = log_softmax(logits)
                lg = eppool.tile([P, F_OUT], f32, tag="lg")
                nc.vector.tensor_add(lg[:], pl[:], b2_sb[:])
                nmx = eppool.tile([P, 1], f32, tag="nmx")
                nc.vector.tensor_reduce(nmx[:], lg[:], axis=mybir.AxisListType.X,
                                        op=mybir.AluOpType.max, negate=True)
                ex = eppool.tile([P, F_OUT], f32, tag="ex")
                nc.scalar.activation(ex[:], lg[:],
                                     mybir.ActivationFunctionType.Exp,
                                     bias=nmx[:])
                sme = eppool.tile([P, 1], f32, tag="sme")
                nc.vector.reduce_sum(sme[:], ex[:], axis=mybir.AxisListType.X)
                lns = eppool.tile([P, 1], f32, tag="lns")
                nc.scalar.activation(lns[:], sme[:],
                                     mybir.ActivationFunctionType.Ln)
                oo = eppool.tile([P, F_OUT], f32, tag="oo")
                nc.vector.tensor_scalar(
                    oo[:], lg[:], nmx[:], lns[:],
                    op0=mybir.AluOpType.add, op1=mybir.AluOpType.subtract)
                nc.sync.dma_start(out.ap()[t0:t0 + tw, :], oo[:tw, :])

    nc.compile()
    return nc


def _prepare(x, edge_row, edge_col, edge_val, W1, b1, W2, b2):
    C_ts, off_ts, TC, idx16, smats = _preprocess(
        np.asarray(edge_row), np.asarray(edge_col), np.asarray(edge_val))
    nc = _build_program(C_ts, off_ts, TC)

    x = np.asarray(x, np.float32)
    W1 = np.asarray(W1, np.float32)
    W2 = np.asarray(W2, np.float32)
    b1 = np.asarray(b1, np.float32)
    b2 = np.asarray(b2, np.float32)

    b1_np = b1.reshape(F_HID, 1).astype(np.float32)
    b2_np = np.broadcast_to(b2[None, :], (P, F_OUT)).copy().astype(np.float32)
    W1_np = W1.astype(BF16)
    W2_np = W2.astype(BF16)

    in_maps = []
    for c in range(NC):
        xk = x[c * SHARD:(c + 1) * SHARD]
        in_maps.append({
            "xT": np.ascontiguousarray(xk.T).astype(BF16),
            "W1b": W1_np, "W2b": W2_np,
            "b1c": b1_np, "b2bc": b2_np,
            "idx16": idx16[c], "smat": smats[c],
        })

    return nc, in_maps


def kernel(x, edge_row, edge_col, edge_val, W1, b1, W2, b2):
    nc, in_maps = _prepare(x, edge_row, edge_col, edge_val, W1, b1, W2, b2)
    res = run_bass_kernel_spmd(nc, in_maps, core_ids=list(range(NC)),
                               trace=False)
    return np.concatenate([res.results[c]["out"] for c in range(NC)], axis=0)



# revision 2
# speedup vs baseline: 1.0125x; 1.0125x over previous
"""GCN (2-layer graph convolution) on 8 TRN2 NeuronCores — v2.

Key facts this design is built around (HW-measured):
  - dma_gather cost ~2.75ns/idx when calls are ~2K idx and rotate strictly
    across the 4 SWDGE queues (calls on different queues run concurrently
    on different Q7 core pairs); single-queue streaks serialize at ~9.5ns.
    256B elems are the minimum (sub-256B is broken+slow); so each phase's
    SpMM gather costs ~1.1ms of Pool time — the kernel's wall.
  - Redundant phase A (every core computes full support1 = x@W1) beats the
    25.6MB AllGather (62GB/s) and removes the barrier.
  - Selection matrices streamed as fp8e4 on HWDGE queues (verified ok,
    mixed-dtype matmul with bf16 gathered features).
  - T2 (h@W2) AllGathered compact [*,32] quarter-wise, locally expanded to
    256B-row padded tables.
  - DVE tensor_scalar with f32 scalar-AP operands is ~1-6us (microcoded);
    epilogue instead uses ACT Identity/Exp with per-partition bias APs
    (~300ns) and batches Exp/Ln to avoid ACT table reloads.
"""

import sys

sys.path.insert(0, "/opt/trn_rl_repo")

import numpy as np
import ml_dtypes

import concourse.bass as bass
import concourse.tile as tile
from concourse import bacc, mybir
from concourse.bass_utils import run_bass_kernel_spmd
from concourse.library_config import mlp

N = 100000
E = 3200000
F_IN, F_HID, F_OUT = 512, 128, 32
NC = 8
SHARD = N // NC          # 12500
P = 128
NT = (SHARD + P - 1) // P   # 98 tiles; last has 84 rows
NSEG = 4
SEG = N // NSEG          # 25000 rows per gather segment (fits int16)
QSEG = SHARD // NSEG     # 3125 rows per core quarter
BF16 = ml_dtypes.bfloat16
FP8 = ml_dtypes.float8_e4m3
G = 5                    # dst tiles per gather group
NGRP = (NT + G - 1) // G  # tile groups per segment
ABLK = 1024              # nodes per phase-A block
SUBCHUNK = 15            # chunks per gather sub-call (~1.9K idx)

USE_FP8_SMAT = True
USE_FP8_A = True
SMAT_NP = FP8 if USE_FP8_SMAT else BF16
XDT_NP = FP8 if USE_FP8_A else BF16


def _preprocess(edge_row, edge_col, edge_val, seg, idx):
    er = edge_row.astype(np.int64)
    k = er // SHARD
    t = (er % SHARD) // P
    key = (k * NSEG + seg) * NT + t
    order = np.argsort(key, kind="stable")
    counts = np.bincount(key, minlength=NC * NSEG * NT).reshape(NC, NSEG, NT)
    C_st = -(-counts.max(axis=0) // P)           # [NSEG, NT]
    TC = int(C_st.sum())
    off_flat = np.concatenate([[0], np.cumsum(C_st.flatten())])[:-1]
    off_st = off_flat.reshape(NSEG, NT)

    starts = np.zeros(NC * NSEG * NT + 1, np.int64)
    starts[1:] = np.cumsum(counts.flatten())
    key_s = key[order]
    rank = np.arange(E, dtype=np.int64) - starts[key_s]
    base = (np.arange(NC)[:, None, None] * TC + off_st[None]) * P
    slot = base.reshape(-1)[key_s] + rank

    idx_slots = np.zeros(NC * TC * P, np.int16)
    rows_slots = np.zeros(NC * TC * P, np.int64)
    vals_slots = np.zeros(NC * TC * P, np.float32)
    ero, evo = edge_row[order], edge_val[order]
    idx_slots[slot] = idx[order].astype(np.int16)
    rows_slots[slot] = (ero.astype(np.int64) % SHARD) % P
    vals_slots[slot] = evo.astype(np.float32)

    idx_slots = idx_slots.reshape(NC, TC * P)
    idx16 = np.stack(
        [np.tile(idx_slots[c].reshape(-1, 16).T, (8, 1)) for c in range(NC)]
    )                                                     # [NC, 128, 8*TC]

    rows_k = rows_slots.reshape(NC, TC * P)
    vals_k = vals_slots.reshape(NC, TC * P)
    smats = np.empty((NC, P, TC * P), dtype=SMAT_NP)
    for c in range(NC):
        S = np.zeros((TC * P, P), dtype=SMAT_NP)
        S[np.arange(TC * P), rows_k[c]] = vals_k[c].astype(SMAT_NP)
        smats[c] = np.ascontiguousarray(
            S.reshape(TC, P, P).transpose(1, 0, 2).reshape(P, TC * P))
    return C_st, off_st, TC, idx16, smats


def _build_program(C_B, off_B, TC_B, C_D, off_D, TC_D):
    f32, bf16, i16 = mybir.dt.float32, mybir.dt.bfloat16, mybir.dt.int16
    smdt = mybir.dt.float8e4 if USE_FP8_SMAT else bf16
    nc = bacc.Bacc("TRN2", target_bir_lowering=False, debug=False,
                   num_devices=NC, num_swdge_queues=4)

    NB = (N + ABLK - 1) // ABLK                   # 98 phase-A blocks

    def group_info(C_st, off_st, s, gg):
        t0 = gg * G
        nt = min(G, NT - t0)
        coff = int(off_st[s, t0])
        cg = int(C_st[s, t0:t0 + nt].sum())
        return t0, nt, coff, cg

    CGmax = 0
    for C_st, off_st in ((C_B, off_B), (C_D, off_D)):
        for s in range(NSEG):
            for gg in range(NGRP):
                CGmax = max(CGmax, group_info(C_st, off_st, s, gg)[3])

    xdt = mybir.dt.float8e4 if USE_FP8_A else bf16
    xT = nc.dram_tensor("xT", [F_IN, N], xdt, kind="ExternalInput")
    W1b = nc.dram_tensor("W1b", [F_IN, F_HID], xdt, kind="ExternalInput")
    W2b = nc.dram_tensor("W2b", [F_HID, F_OUT], bf16, kind="ExternalInput")
    b1c = nc.dram_tensor("b1c", [P, 1], f32, kind="ExternalInput")
    b2bc = nc.dram_tensor("b2bc", [P, F_OUT], f32, kind="ExternalInput")
    idxB = nc.dram_tensor("idxB", [P, 8 * TC_B], i16, kind="ExternalInput")
    smB = nc.dram_tensor("smB", [P, TC_B * P], smdt, kind="ExternalInput")
    idxD = nc.dram_tensor("idxD", [P, 8 * TC_D], i16, kind="ExternalInput")
    smD = nc.dram_tensor("smD", [P, TC_D * P], smdt, kind="ExternalInput")
    out = nc.dram_tensor("out", [SHARD, F_OUT], f32, kind="ExternalOutput")

    T1 = nc.dram_tensor("T1", [N, F_HID], bf16)
    T2_local = nc.dram_tensor("T2_local", [SHARD, F_OUT], bf16)
    T2c = [nc.dram_tensor(f"T2c{s}", [196 * P, F_OUT], bf16,
                          addr_space="Shared") for s in range(NSEG)]
    T2p = [nc.dram_tensor(f"T2p{s}", [196 * P, P], bf16) for s in range(NSEG)]

    C_Bt = C_B.sum(axis=0)
    first_D = [int(np.argmax(C_D[:, t] > 0)) for t in range(NT)]
    last_D = [NSEG - 1 - int(np.argmax(C_D[::-1, t] > 0)) for t in range(NT)]
    ag_after = {}
    for q in range(NSEG):
        tq = ((q + 1) * QSEG - 1) // P
        tb = min(NT - 1, (tq // 4) * 4 + 3)
        ag_after[tb] = q

    qctr = [0]                                    # global gather queue rr

    def gather_sub(gt, table_ap, ix, cg, elem):
        """Issue the group's gather as ~SUBCHUNK-chunk calls, rr queues."""
        c0 = 0
        while c0 < cg:
            cs = min(SUBCHUNK, cg - c0)
            nc.gpsimd.dma_gather(
                gt[:, c0:c0 + cs, :], table_ap,
                ix[:, 8 * c0:8 * (c0 + cs)], cs * P, cs * P, elem,
                single_packet=False, queue_num=qctr[0] % 4)
            qctr[0] += 1
            c0 += cs

    with tile.TileContext(nc) as tc:
        with (
            tc.tile_pool(name="const", bufs=1) as cpool,
            tc.tile_pool(name="xa", bufs=3) as xapool,
            tc.tile_pool(name="s1", bufs=3) as s1pool,
            tc.tile_pool(name="g", bufs=7) as gpool,
            tc.tile_pool(name="sm", bufs=5) as smpool,
            tc.tile_pool(name="ix", bufs=7) as ixpool,
            tc.tile_pool(name="acc", bufs=1) as accpool,
            tc.tile_pool(name="exp", bufs=2) as xppool,
            tc.tile_pool(name="ep", bufs=4) as eppool,
            tc.tile_pool(name="st", bufs=3) as stpool,
            tc.tile_pool(name="pp", bufs=4, space="PSUM") as pppool,
            tc.tile_pool(name="ps", bufs=2, space="PSUM") as pspool,
            tc.tile_pool(name="pl", bufs=2, space="PSUM") as plpool,
        ):
            nc.gpsimd.load_library(mlp)

            # ---- resident constants ----
            W1_sb = cpool.tile([P, 4, F_HID], xdt)
            nc.sync.dma_start(
                W1_sb[:], W1b.ap().rearrange("(kk p) f -> p kk f", p=P))
            W2_sb = cpool.tile([P, F_OUT], bf16)
            nc.sync.dma_start(W2_sb[:], W2b.ap())
            b1_sb = cpool.tile([P, 1], f32)
            nc.sync.dma_start(b1_sb[:], b1c.ap())
            b2_sb = cpool.tile([P, F_OUT], f32)
            nc.sync.dma_start(b2_sb[:], b2bc.ap())

            # ---- phase A: T1 = x @ W1 for ALL nodes (redundant) ----
            for b in range(NB):
                m0 = b * ABLK
                mw = min(ABLK, N - m0)             # 1024 or 672
                nsub = (mw + P - 1) // P
                xa = xapool.tile([P, 4, ABLK], xdt, tag="xa")
                ldq = nc.sync if b % 2 == 0 else nc.scalar
                ldq.dma_start(
                    xa[:, :, :mw],
                    xT.ap()[:, m0:m0 + mw].rearrange("(kk p) m -> p kk m", p=P))
                s1 = s1pool.tile([P, nsub, F_HID], bf16, tag="s1",
                                 padded_shape=[P, 8, F_HID])
                for j in range(nsub):
                    jw = min(P, mw - j * P)
                    ps = pppool.tile([P, F_HID], f32, tag="ph", space="PSUM")
                    if USE_FP8_A:
                        for kk in range(2):
                            nc.tensor.matmul(
                                ps[:jw, :],
                                xa[:, 2 * kk:2 * kk + 2, j * P:j * P + jw],
                                W1_sb[:, 2 * kk:2 * kk + 2, :],
                                start=(kk == 0), stop=(kk == 1),
                                perf_mode=mybir.MatmulPerfMode.DoubleRow)
                    else:
                        for kk in range(4):
                            nc.tensor.matmul(
                                ps[:jw, :], xa[:, kk, j * P:j * P + jw],
                                W1_sb[:, kk, :], start=(kk == 0),
                                stop=(kk == 3))
                    if j % 2 == 0:
                        nc.scalar.activation(
                            s1[:jw, j, :], ps[:jw, :],
                            mybir.ActivationFunctionType.Copy)
                    else:
                        nc.vector.tensor_copy(s1[:jw, j, :], ps[:jw, :])
                nfull = mw // P
                wq = nc.sync if b % 2 == 1 else nc.scalar
                if nfull:
                    wq.dma_start(
                        T1.ap()[m0:m0 + nfull * P, :].rearrange(
                            "(bb p) f -> p bb f", p=P),
                        s1[:, :nfull, :])
                if mw % P:
                    wq.dma_start(
                        T1.ap()[m0 + nfull * P:m0 + mw, :],
                        s1[:mw % P, nfull, :])

            # ---- phase B: SpMM1 + relu + @W2 -> T2_local (compact) ----
            bufsB = {}

            def fetchB(s, gg):
                t0, nt, coff, cg = group_info(C_B, off_B, s, gg)
                ix = ixpool.tile([P, 8 * CGmax], i16, tag="ix", name="ix")
                nc.scalar.dma_start(ix[:, :8 * cg],
                                    idxB.ap()[:, 8 * coff:8 * (coff + cg)])
                gt = gpool.tile([P, CGmax, P], bf16, tag="g", name="gt")
                gather_sub(gt, T1.ap()[SEG * s:SEG * (s + 1), :], ix, cg,
                           F_HID)
                sm = smpool.tile([P, CGmax * P], smdt, tag="sm", name="sm")
                nc.scalar.dma_start(sm[:, :cg * P],
                                    smB.ap()[:, coff * P:(coff + cg) * P])
                bufsB[(s, gg)] = (gt, sm, coff)

            fetched = 0
            s2stage = None
            for t in range(NT):
                gg = t // G
                while fetched < NGRP * NSEG and fetched < (gg + 2) * NSEG:
                    fetchB(fetched % NSEG, fetched // NSEG)
                    fetched += 1
                ph = pppool.tile([P, P], f32, tag="ph", space="PSUM")
                ct = int(C_Bt[t])
                if ct == 0:
                    nc.vector.memset(ph[:], 0.0)
                ci = 0
                for s in range(NSEG):
                    cs = int(C_B[s, t])
                    if cs == 0:
                        continue
                    gt, sm, coff = bufsB[(s, gg)]
                    ro = int(off_B[s, t]) - coff
                    for c in range(cs):
                        nc.tensor.matmul(
                            ph[:], gt[:, ro + c, :],
                            sm[:, (ro + c) * P:(ro + c + 1) * P],
                            start=(ci == 0), stop=(ci == ct - 1))
                        ci += 1
                hT = eppool.tile([P, P], bf16, tag="hT")
                nc.scalar.activation(hT[:], ph[:],
                                     mybir.ActivationFunctionType.Relu,
                                     bias=b1_sb[:])
                ps2 = pspool.tile([P, F_OUT], f32, tag="ps2", space="PSUM")
                nc.tensor.matmul(ps2[:], hT[:], W2_sb[:], start=True, stop=True)
                if t % 4 == 0:
                    s2stage = stpool.tile([P, 4, F_OUT], bf16, tag="s2",
                                          name="s2stage")
                nc.vector.tensor_copy(s2stage[:, t % 4, :], ps2[:])
                if t % 4 == 3 or t == NT - 1:
                    r0 = (t // 4) * 4 * P
                    r1 = min(SHARD, (t + 1) * P)
                    nf = (r1 - r0) // P
                    if nf:
                        nc.sync.dma_start(
                            T2_local.ap()[r0:r0 + nf * P, :].rearrange(
                                "(bb p) f -> p bb f", p=P),
                            s2stage[:, :nf, :])
                    if (r1 - r0) % P:
                        nc.sync.dma_start(
                            T2_local.ap()[r0 + nf * P:r1, :],
                            s2stage[:(r1 - r0) % P, nf, :])
                    if t in ag_after:
                        q = ag_after[t]
                        nc.gpsimd.collective_compute(
                            "AllGather", mybir.AluOpType.bypass,
                            replica_groups=[list(range(NC))],
                            ins=[T2_local.ap()[q * QSEG:(q + 1) * QSEG,
                                               :].opt()],
                            outs=[T2c[q].ap()[:SEG, :].opt()],
                        )

            # ---- expand compact T2 segments to 256B-row padded tables ----
            for s in range(NSEG):
                for sub in range(4):
                    r0 = sub * 49 * P
                    cin = xppool.tile([P, 49, F_OUT], bf16, tag="cin",
                                      name="cin")
                    nc.sync.dma_start(
                        cin[:],
                        T2c[s].ap()[r0:r0 + 49 * P, :].rearrange(
                            "(bb p) f -> p bb f", p=P))
                    cout = xppool.tile([P, 49, P], bf16, tag="cout", bufs=1,
                                       name="cout")
                    nc.vector.tensor_copy(cout[:, :, :F_OUT], cin[:])
                    nc.sync.dma_start(
                        T2p[s].ap()[r0:r0 + 49 * P, :].rearrange(
                            "(bb p) f -> p bb f", p=P),
                        cout[:])

            # ---- phase D: SpMM2 seg-major, SBUF acc, inline epilogue ----
            accs = [accpool.tile([P, F_OUT], f32, tag=f"acc{t}", bufs=1,
                                 name=f"acc{t}") for t in range(NT)]
            for t in range(NT):
                if int(C_D[:, t].sum()) == 0:
                    nc.vector.memset(accs[t][:], 0.0)

            def epilogue(tiles):
                """+b2, log_softmax, write out — batched for ACT table."""
                st1 = []
                for t in tiles:
                    lg = eppool.tile([P, F_OUT], f32, tag="lg", bufs=8,
                                     name="lg")
                    nc.vector.tensor_add(lg[:], accs[t][:], b2_sb[:])
                    nmx = eppool.tile([P, 1], f32, tag="nmx", bufs=8,
                                      name="nmx")
                    nc.vector.tensor_reduce(nmx[:], lg[:],
                                            axis=mybir.AxisListType.X,
                                            op=mybir.AluOpType.max,
                                            negate=True)
                    st1.append((t, lg, nmx))
                st2 = []
                for t, lg, nmx in st1:
                    ex = eppool.tile([P, F_OUT], f32, tag="ex", bufs=8,
                                     name="ex")
                    nc.scalar.activation(ex[:], lg[:],
                                         mybir.ActivationFunctionType.Exp,
                                         bias=nmx[:])
                    sme = eppool.tile([P, 1], f32, tag="sme", bufs=8,
                                      name="sme")
                    nc.vector.reduce_sum(sme[:], ex[:],
                                         axis=mybir.AxisListType.X)
                    st2.append((t, lg, nmx, sme))
                st3 = []
                for t, lg, nmx, sme in st2:
                    ln = eppool.tile([P, 1], f32, tag="lns", bufs=8,
                                     name="ln")
                    nc.scalar.activation(ln[:], sme[:],
                                         mybir.ActivationFunctionType.Ln)
                    nb = eppool.tile([P, 1], f32, tag="nb", bufs=8,
                                     name="nb")
                    nc.vector.tensor_sub(nb[:], nmx[:], ln[:])
                    st3.append((t, lg, nb))
                for t, lg, nb in st3:
                    oo = eppool.tile([P, F_OUT], f32, tag="oo", bufs=8,
                                     name="oo")
                    nc.scalar.activation(oo[:], lg[:],
                                         mybir.ActivationFunctionType.Identity,
                                         bias=nb[:])
                    tw = min(P, SHARD - t * P)
                    nc.sync.dma_start(out.ap()[t * P:t * P + tw, :],
                                      oo[:tw, :])

            for s in range(NSEG):
                for gg in range(NGRP):
                    t0, nt, coff, cg = group_info(C_D, off_D, s, gg)
                    ix = ixpool.tile([P, 8 * CGmax], i16, tag="ix", name="ix")
                    nc.scalar.dma_start(
                        ix[:, :8 * cg], idxD.ap()[:, 8 * coff:8 * (coff + cg)])
                    gt = gpool.tile([P, CGmax, P], bf16, tag="g", name="gt")
                    gather_sub(gt, T2p[s].ap()[:SEG, :], ix, cg, P)
                    sm = smpool.tile([P, CGmax * P], smdt, tag="sm", name="sm")
                    nc.scalar.dma_start(
                        sm[:, :cg * P], smD.ap()[:, coff * P:(coff + cg) * P])
                    done = []
                    for t in range(t0, t0 + nt):
                        cs = int(C_D[s, t])
                        if cs == 0:
                            if s == last_D[t]:
                                done.append(t)
                            continue
                        ro = int(off_D[s, t]) - coff
                        pl = plpool.tile([P, F_OUT], f32, tag="pl",
                                         space="PSUM")
                        for c in range(cs):
                            nc.tensor.matmul(
                                pl[:], sm[:, (ro + c) * P:(ro + c + 1) * P],
                                gt[:, ro + c, :F_OUT],
                                start=(c == 0), stop=(c == cs - 1))
                        if s == first_D[t]:
                            nc.vector.tensor_copy(accs[t][:], pl[:])
                        else:
                            nc.vector.tensor_add(accs[t][:], accs[t][:],
                                                 pl[:])
                        if s == last_D[t]:
                            done.append(t)
                    if done:
                        epilogue(done)

    nc.compile()
    return nc


def _prepare(x, edge_row, edge_col, edge_val, W1, b1, W2, b2):
    er = np.asarray(edge_row)
    ec = np.asarray(edge_col).astype(np.int64)
    ev = np.asarray(edge_val)

    segB = ec // SEG
    idxB = ec % SEG
    C_B, off_B, TC_B, idx16B, smB = _preprocess(er, ec, ev, segB, idxB)

    segD = (ec % SHARD) // QSEG
    idxD = (ec // SHARD) * QSEG + (ec % QSEG)
    C_D, off_D, TC_D, idx16D, smD = _preprocess(er, ec, ev, segD, idxD)

    nc = _build_program(C_B, off_B, TC_B, C_D, off_D, TC_D)

    x = np.asarray(x, np.float32)
    W1 = np.asarray(W1, np.float32)
    W2 = np.asarray(W2, np.float32)
    b1 = np.asarray(b1, np.float32)
    b2 = np.asarray(b2, np.float32)

    xT_np = np.ascontiguousarray(x.T).astype(XDT_NP)
    W1_np = W1.astype(XDT_NP)
    W2_np = W2.astype(BF16)
    b1_np = b1.reshape(F_HID, 1).astype(np.float32)
    b2_np = np.broadcast_to(b2[None, :], (P, F_OUT)).copy().astype(np.float32)

    in_maps = []
    for c in range(NC):
        in_maps.append({
            "xT": xT_np, "W1b": W1_np, "W2b": W2_np,
            "b1c": b1_np, "b2bc": b2_np,
            "idxB": idx16B[c], "smB": smB[c],
            "idxD": idx16D[c], "smD": smD[c],
        })
    return nc, in_maps


def kernel(x, edge_row, edge_col, edge_val, W1, b1, W2, b2):
    nc, in_maps = _prepare(x, edge_row, edge_col, edge_val, W1, b1, W2, b2)
    res = run_bass_kernel_spmd(nc, in_maps, core_ids=list(range(NC)),
                               trace=False)
    return np.concatenate([res.results[c]["out"] for c in range(NC)], axis=0)
